# revision 21
# baseline (speedup 1.0000x reference)
"""Distance-aware transformer encoder layer on 8 Trainium2 NeuronCores.

Sharding: core c handles batch b = c//2 and query-half qh = c%2 (512 query
rows). K/V are computed per-core for the full 1024-key sequence of its batch
(duplicated across the core pair — cheaper than a collective). Everything
else (scores, softmax, out-proj, LayerNorms, FFN) is perfectly sharded by
query rows. No collectives.

v2 changes vs baseline (653us):
- distance bias folded multiplicatively: E = exp(scores) * P with
  P = (d+1e-9)^(-|dist_scale|) precomputed HOST-side and DMA'd in bf16.
  Removes the 65K-row ident@biasT PE matmuls and the on-chip ln/exp chain
  that serialized the startup DMA queue.
- all weights shipped bf16 and host-relaid so every DMA is 128 contiguous
  per-partition runs (no descriptor storms).
- two HW DMA queues: SP carries the critical stream (srcT, Wk/Wq/Wv, P,
  src_q, W1, out), Act carries never-blocking prefetches (LN params, Wo, W2).
- attention pipelined per-head (scores / exp / P-mult / attnV overlap), with
  the second half of the QKV projections interleaved between heads to keep
  the PE dense while Act does exp.
- softmax denominator reciprocal via DVE reciprocal_approx_fast (the exact
  `reciprocal` cost 2.3us/head) broadcast across partitions with a K=1
  ones-outer-product matmul instead of a DRAM round-trip.
- LayerNorm rstd via single Rsqrt activation.
- FFN2 streams W2 in 4 column-quarter tiles (bufs=2) so LN2 + output DMA
  pipeline behind the last pass.
"""

import numpy as np
import ml_dtypes

import bass_rust
import concourse.bass as bass
import concourse.tile as tile
import concourse.mybir as mybir
from concourse.bass import AP
from concourse.bass_utils import run_bass_kernel_spmd

B, S, D, H, DFF, HD = 4, 1024, 1024, 16, 4096, 64
SQ = 512          # query rows per core
NCORES = 8
EPS = 1e-5
F32 = mybir.dt.float32
F32R = mybir.dt.float32r
BF16 = mybir.dt.bfloat16
FT = mybir.ActivationFunctionType
ALU = mybir.AluOpType

SPIN = 64

_nop_ctr = [0]


def _legalize_waits(nc):
    """walrus codegen in this toolchain accepts only one sync-wait per
    instruction; split extras onto same-engine NoOps inserted before."""
    n_fixed = 0
    for f in nc.m.functions:
        for bb in f.blocks:
            insts = bb.instructions
            i = 0
            while i < len(insts):
                inst = insts[i]
                si = inst.sync_info
                waits = list(si.on_wait) if si is not None and si.on_wait else []
                if len(waits) > 1:
                    keep = waits[-1]
                    for w in waits[:-1]:
                        n = bass_rust.InstNoOp(
                            name=f"waitsplit-nop-{_nop_ctr[0]}", ins=[], outs=[]
                        )
                        _nop_ctr[0] += 1
                        n.engine = inst.engine
                        n.sync_info = bass_rust.SyncInfo(on_update=[], on_wait=[w])
                        insts.insert(i, n)
                        i += 1
                    inst.sync_info = bass_rust.SyncInfo(
                        on_update=list(si.on_update or []), on_wait=[keep]
                    )
                    n_fixed += 1
                i += 1
    return n_fixed


def _build():
    nc = bass.Bass()
    dp = nc.declare_dram_parameter

    SrcT = dp("srcT", [128, 8, S], BF16, isOutput=False)    # [ki, ko, s]
    SrcQ = dp("src_q", [128, 4, D], F32, isOutput=False)    # [ki, qt, d] (+bo)
    PT = dp("pt", [128, 8, SQ], BF16, isOutput=False)       # P[k,q] [ki,ko,q]
    Wk = dp("wk", [4, 128, 8, 256], BF16, isOutput=False)   # [wc][ki][ko][dout]
    Wq = dp("wq", [4, 128, 8, 256], BF16, isOutput=False)   # pre-scaled HD^-.5
    Wv = dp("wv", [2, 128, 8, 512], BF16, isOutput=False)   # [vc2][ki][ko][dout]
    Wo = dp("wo", [128, 8, D], BF16, isOutput=False)        # [ki][dp][dout]
    W1 = dp("w1", [16, 128, 8, 256], BF16, isOutput=False)  # [fc][ki][co][f]
    W2 = dp("w2", [4, 128, 32, 256], BF16, isOutput=False)  # [pq][fi][fo][d]
    C3 = dp("c3", [128, 48], F32, isOutput=False)           # bq2|bk2|b12
    BvR = dp("bv_r", [1, D], F32R, isOutput=False)
    B2R = dp("b2_r", [1, D], F32R, isOutput=False)
    G1 = dp("g1b", [128, D], BF16, isOutput=False)
    Be1 = dp("beta1b", [128, D], BF16, isOutput=False)
    G2 = dp("g2b", [128, D], BF16, isOutput=False)
    Be2 = dp("beta2b", [128, D], BF16, isOutput=False)
    Ident = dp("ident", [128, 128], F32R, isOutput=False)
    OnesR = dp("ones_row", [1, 512], F32R, isOutput=False)
    Out = dp("out", [SQ, D], F32, isOutput=True)

    with tile.TileContext(nc) as tc:
        import contextlib

        ctx = contextlib.ExitStack()
        with ctx:
            consts = ctx.enter_context(tc.tile_pool(name="consts", bufs=1))
            ao_pool = ctx.enter_context(tc.tile_pool(name="ao_pool", bufs=1))
            wo_pool = ctx.enter_context(tc.tile_pool(name="wo_pool", bufs=1))
            srcq_pool = ctx.enter_context(tc.tile_pool(name="srcq_pool", bufs=1))
            small = ctx.enter_context(tc.tile_pool(name="small", bufs=2))
            ln_small = ctx.enter_context(tc.tile_pool(name="ln_small", bufs=4))
            rsb_pool = ctx.enter_context(tc.tile_pool(name="rsb", bufs=2))
            stash_pool = ctx.enter_context(tc.tile_pool(name="stash", bufs=4))
            ps_a = ctx.enter_context(tc.tile_pool(name="ps_a", bufs=2, space="PSUM"))
            ps_b = ctx.enter_context(tc.tile_pool(name="ps_b", bufs=4, space="PSUM"))

            # ---- consts: only what the first K/Q matmuls need goes first ----
            c3 = consts.tile([128, 48], F32, tag="c3")
            nc.sync.dma_start(out=c3, in_=C3[:, :])
            bq2 = c3[:, 0:8]
            bk2 = c3[:, 8:16]
            b12 = c3[:, 16:48]
            ident = consts.tile([128, 128], F32R, tag="ident")
            ones_row = consts.tile([1, 512], F32R, tag="ones_row")
            bv_r = consts.tile([1, D], F32R, tag="bv_r")
            b2_r = consts.tile([1, D], F32R, tag="b2_r")

            # ---- prefetches on Act queue (never block) ----
            g1b = consts.tile([128, D], BF16, tag="g1b")
            nc.scalar.dma_start(out=g1b, in_=G1[:, :])
            be1b = consts.tile([128, D], BF16, tag="be1b")
            nc.scalar.dma_start(out=be1b, in_=Be1[:, :])
            g2b = consts.tile([128, D], BF16, tag="g2b")
            nc.scalar.dma_start(out=g2b, in_=G2[:, :])
            be2b = consts.tile([128, D], BF16, tag="be2b")
            nc.scalar.dma_start(out=be2b, in_=Be2[:, :])
            wo_sb = wo_pool.tile([128, 8, D], BF16, tag="wo")
            nc.scalar.dma_start(out=wo_sb, in_=Wo[:, :, :])

            ln_eps = consts.tile([128, 1], F32, tag="ln_eps")
            nc.vector.memset(ln_eps, EPS)
            spin_src = consts.tile([128, 512], BF16, tag="spin_src")
            nc.vector.memset(spin_src, 0.0)

            # ---- spin: ramp the PE while the first DMAs land ----
            for _ in range(SPIN):
                sp = ps_b.tile([128, 512], F32, tag="psb")
                nc.tensor.matmul(sp, spin_src[:, 0:128], spin_src, start=True, stop=True)

            # ---- persistent activations ----
            ao_sb = ao_pool.tile([128, 8, 512], BF16, tag="ao")
            src_q = srcq_pool.tile([128, 4, D], F32, tag="srcq")

            with tc.tile_pool(name="attn", bufs=1) as attn_pool, \
                 tc.tile_pool(name="wkq", bufs=2) as wkq_pool, \
                 tc.tile_pool(name="wv2", bufs=2) as wv_pool, \
                 tc.tile_pool(name="epool", bufs=6) as epool:

                srcT = attn_pool.tile([128, 8, S], BF16, tag="srcT")
                nc.sync.dma_start(out=srcT[:, 0:4, :], in_=SrcT[:, 0:4, :])
                nc.scalar.dma_start(out=srcT[:, 4:8, :], in_=SrcT[:, 4:8, :])
                kt = attn_pool.tile([128, 8, S], BF16, tag="kt")
                qt = attn_pool.tile([128, 8, SQ], BF16, tag="qt")
                p_sb = attn_pool.tile([128, 8, SQ], BF16, tag="p_sb")
                v_sb = attn_pool.tile([128, 8, 16, 65], BF16, tag="v_sb")
                nc.vector.memset(v_sb[:, :, :, 64:65], 1.0)

                _kbufs, _qbufs = {}, {}

                def emit_k_chunk(wc, groups=None):
                    if wc not in _kbufs:
                        wb = wkq_pool.tile(
                            [128, 8, 256], BF16, tag="wkq", name=f"wkb{wc}"
                        )
                        _kbufs[wc] = wb
                        nc.sync.dma_start(out=wb, in_=Wk[wc, :, :, :])
                    wb = _kbufs[wc]
                    for g in range(4) if groups is None else groups:
                        dl, nt = g // 2, g % 2
                        dt = wc * 2 + dl
                        psum = ps_b.tile([128, 512], F32, tag="psb")
                        for ko in range(8):
                            nc.tensor.matmul(
                                psum,
                                wb[:, ko, dl * 128 : dl * 128 + 128],
                                srcT[:, ko, nt * 512 : nt * 512 + 512],
                                start=(ko == 0),
                                stop=(ko == 7),
                            )
                        nc.vector.tensor_scalar_add(
                            out=kt[:, dt, nt * 512 : nt * 512 + 512],
                            in0=psum,
                            scalar1=bk2[:, dt : dt + 1],
                        )

                def emit_q_chunk(wc, groups=None):
                    if wc not in _qbufs:
                        wb = wkq_pool.tile(
                            [128, 8, 256], BF16, tag="wkq", name=f"wqb{wc}"
                        )
                        _qbufs[wc] = wb
                        nc.sync.dma_start(out=wb, in_=Wq[wc, :, :, :])
                    wb = _qbufs[wc]
                    for dl in range(2) if groups is None else groups:
                        dt = wc * 2 + dl
                        psum = ps_b.tile([128, 512], F32, tag="psb")
                        for ko in range(8):
                            nc.tensor.matmul(
                                psum,
                                wb[:, ko, dl * 128 : dl * 128 + 128],
                                srcT[:, ko, 0:512],
                                start=(ko == 0),
                                stop=(ko == 7),
                            )
                        nc.vector.tensor_scalar_add(
                            out=qt[:, dt, :], in0=psum, scalar1=bq2[:, dt : dt + 1]
                        )

                def emit_v_chunk(vc2, mts, load):
                    if load:
                        wb = wv_pool.tile(
                            [128, 8, 512], BF16, tag="wv", name=f"wvb{vc2}"
                        )
                        emit_v_chunk.wb[vc2] = wb
                        nc.sync.dma_start(out=wb, in_=Wv[vc2, :, :, :])
                    wb = emit_v_chunk.wb[vc2]
                    for mt in mts:
                        psum = ps_b.tile([128, 512], F32, tag="psb")
                        nc.tensor.matmul(
                            psum,
                            ones_row[0:1, 0:128],
                            bv_r[0:1, vc2 * 512 : vc2 * 512 + 512],
                            start=True,
                            stop=False,
                        )
                        for ko in range(8):
                            nc.tensor.matmul(
                                psum,
                                srcT[:, ko, mt * 128 : mt * 128 + 128],
                                wb[:, ko, :],
                                start=False,
                                stop=(ko == 7),
                            )
                        nc.vector.tensor_copy(
                            out=v_sb[:, mt, vc2 * 8 : vc2 * 8 + 8, 0:64],
                            in_=psum.rearrange("p (h e) -> p h e", e=64),
                        )

                emit_v_chunk.wb = {}

                # ---- group 0: everything heads 0-7 need ----
                emit_k_chunk(0)
                emit_q_chunk(0)
                emit_k_chunk(1)
                emit_q_chunk(1)
                # V-bias consts must beat the group-0 V matmuls; they are tiny
                nc.sync.dma_start(out=ones_row, in_=OnesR[:, :])
                nc.sync.dma_start(out=bv_r, in_=BvR[:, :])
                nc.sync.dma_start(out=b2_r, in_=B2R[:, :])
                nc.sync.dma_start(out=ident, in_=Ident[:, :])
                emit_v_chunk(0, range(8), load=True)
                # P + src_q loads behind group-0 weights on SP
                nc.sync.dma_start(out=p_sb, in_=PT[:, :, :])
                nc.sync.dma_start(out=src_q, in_=SrcQ[:, :, :])

                # group-1 work interleaved between attention heads at
                # single-psum-group granularity (bounded PSUM footprint)
                def k_units(wc):
                    units = []
                    for i in range(4):
                        units.append(lambda wc=wc, i=i: emit_k_chunk(wc, [i]))
                    return units

                def q_units(wc):
                    return [
                        lambda wc=wc: emit_q_chunk(wc, [0]),
                        lambda wc=wc: emit_q_chunk(wc, [1]),
                    ]

                g1_units = (
                    k_units(2)
                    + q_units(2)
                    + [lambda mt=mt: emit_v_chunk(1, [mt], load=(mt == 0))
                       for mt in range(8)]
                    + k_units(3)
                    + q_units(3)
                )
                # units per head-iteration; deadlines: K2/Q2 by iter 7,
                # V1 by iter 8, K3/Q3 by iter 11. PSUM allows 2/iter
                # (4 at iter 0 where no pao/pr tiles are live).
                g1_sched = [2, 2, 2, 2, 2, 1, 1, 1, 1, 2, 2, 2, 0, 0, 0, 0]

                # ---- attention, per-head pipeline ----
                e_tiles = {}
                rec_r = {}
                pao = {}

                def emit_scores_kog(h, kog):
                    base, dt = (h % 2) * 64, h // 2
                    pss = ps_a.tile([128, 2, 512], F32, tag="pss")
                    for kl in range(2):
                        ko = kog * 2 + kl
                        nc.tensor.matmul(
                            pss[:, kl, :],
                            kt[base : base + 64, dt, ko * 128 : ko * 128 + 128],
                            qt[base : base + 64, dt, :],
                            start=True,
                            stop=True,
                        )
                    e_t = epool.tile([128, 2, 512], BF16, tag="e_t")
                    nc.scalar.activation(out=e_t, in_=pss, func=FT.Exp)
                    nc.vector.tensor_mul(
                        out=e_t, in0=e_t, in1=p_sb[:, kog * 2 : kog * 2 + 2, :]
                    )
                    e_tiles[(h, kog)] = e_t

                def emit_attnv(h):
                    pa = ps_b.tile([128, 512], F32, tag="psb")
                    pao[h] = pa
                    for ko in range(8):
                        nc.tensor.matmul(
                            pa[0:65, :],
                            v_sb[:, ko, h, :],
                            e_tiles[(h, ko // 2)][:, ko % 2, :],
                            start=(ko == 0),
                            stop=(ko == 7),
                        )
                    for kog in range(4):
                        del e_tiles[(h, kog)]

                def emit_rec(h):
                    # 1/denom as Exp(-Ln(denom)) on Act: avoids the 2.3us
                    # exact DVE reciprocal and custom-ISA approx ops. For the
                    # last heads (no proj filler; Act is the bottleneck) use
                    # the exact DVE reciprocal instead to rebalance engines.
                    rr = small.tile([1, 512], F32R, tag="rec_r")
                    with nc.allow_low_precision(reason="f32r is f32-bit"):
                        nc.scalar.activation(
                            out=rr, in_=pao[h][64:65, :], func=FT.Ln
                        )
                        nc.scalar.activation(
                            out=rr, in_=rr, func=FT.Exp, scale=-1.0
                        )
                    rec_r[h] = rr

                def emit_norm(h):
                    base, dt = (h % 2) * 64, h // 2
                    pr = ps_b.tile([128, 512], F32, tag="psb")
                    nc.tensor.matmul(
                        pr[0:64, :], ones_row[0:1, 0:64], rec_r[h][0:1, :],
                        start=True, stop=True,
                    )
                    rec_sb = rsb_pool.tile([64, 512], F32, tag="rec_sb")
                    nc.vector.tensor_copy(out=rec_sb, in_=pr[0:64, :])
                    nc.vector.tensor_mul(
                        out=ao_sb[base : base + 64, dt, :],
                        in0=pao[h][0:64, :],
                        in1=rec_sb,
                    )
                    del pao[h], rec_r[h]

                # partial out-projection (heads 0-11 contribution) emitted
                # into the attention tail, where Act/DVE otherwise rate-limit
                # the PE. Results stash to SBUF; dpi 6-7 finish after norms.
                op_stash = {}

                def emit_op_part(qt_i, nt):
                    psum = ps_b.tile(
                        [128, 512], F32, tag="psb", name=f"oppp{qt_i}{nt}"
                    )
                    for j, dpi in enumerate(range(6)):
                        nc.tensor.matmul(
                            psum,
                            ao_sb[:, dpi, qt_i * 128 : qt_i * 128 + 128],
                            wo_sb[:, dpi, nt * 512 : nt * 512 + 512],
                            start=(j == 0),
                            stop=(j == 5),
                        )
                    st = stash_pool.tile(
                        [128, 512], BF16, tag="opst", name=f"opst{qt_i}{nt}"
                    )
                    nc.vector.tensor_copy(out=st, in_=psum)
                    op_stash[(qt_i, nt)] = st

                ui = 0
                for h in range(H):
                    emit_scores_kog(h, 0)
                    emit_scores_kog(h, 1)
                    if h >= 1:
                        emit_attnv(h - 1)
                        emit_rec(h - 1)
                    emit_scores_kog(h, 2)
                    emit_scores_kog(h, 3)
                    n_units = g1_sched[h] if h > 0 else 4
                    for _ in range(n_units):
                        if ui < len(g1_units):
                            g1_units[ui]()
                            ui += 1
                    if h >= 1:
                        emit_norm(h - 1)
                    if h == 13:
                        emit_op_part(0, 0)
                    elif h == 14:
                        emit_op_part(0, 1)
                    elif h == 15:
                        emit_op_part(1, 0)
                emit_attnv(15)
                emit_op_part(1, 1)
                emit_rec(15)
                emit_norm(15)

            # ================= scope 2: out-proj / LN1 / FFN =================
            with tc.tile_pool(name="xpool", bufs=1) as xpool, \
                 tc.tile_pool(name="hpool", bufs=1) as hpool, \
                 tc.tile_pool(name="w1p", bufs=3) as w1p, \
                 tc.tile_pool(name="w2p", bufs=2) as w2p, \
                 tc.tile_pool(name="lnpool", bufs=2) as lnpool:

                x_sb = xpool.tile([128, 4, D], F32R, tag="x")      # xpre then x
                ypre = xpool.tile([128, 4, D], F32, tag="ypre")
                xT = xpool.tile([128, 8, 512], BF16, tag="xT")

                # W2 first two column-quarters prefetch on the Act queue now,
                # so the transfers overlap out-proj/LN1/FFN1.
                w2_tiles = {}
                for pq in range(2):
                    w2_tiles[pq] = w2p.tile(
                        [128, 32, 256], BF16, tag="w2", name=f"w2t{pq}"
                    )
                    nc.scalar.dma_start(out=w2_tiles[pq], in_=W2[pq, :, :, :])

                def emit_outproj(qt_i):
                    for nt in range(2):
                        st = op_stash.pop((qt_i, nt), None)
                        psum = ps_b.tile([128, 512], F32, tag="psb")
                        dpis = range(6, 8) if st is not None else range(8)
                        for j, dpi in enumerate(dpis):
                            nc.tensor.matmul(
                                psum,
                                ao_sb[:, dpi, qt_i * 128 : qt_i * 128 + 128],
                                wo_sb[:, dpi, nt * 512 : nt * 512 + 512],
                                start=(j == 0),
                                stop=(dpi == 7),
                            )
                        xs = x_sb[:, qt_i, nt * 512 : nt * 512 + 512]
                        if st is not None:
                            nc.vector.tensor_add(out=xs, in0=psum, in1=st)
                            nc.vector.tensor_add(
                                out=xs,
                                in0=xs,
                                in1=src_q[:, qt_i, nt * 512 : nt * 512 + 512],
                            )
                        else:
                            nc.vector.tensor_add(
                                out=xs,
                                in0=psum,
                                in1=src_q[:, qt_i, nt * 512 : nt * 512 + 512],
                            )

                ln_ctx = {}

                def emit_ln1_stats(qt_i):
                    xpre = x_sb[:, qt_i, :]
                    stats = ln_small.tile([128, 2, 6], F32, tag="stats")
                    for half in range(2):
                        nc.vector.bn_stats(
                            out=stats[:, half, :],
                            in_=xpre[:, half * 512 : half * 512 + 512],
                        )
                    mv = ln_small.tile([128, 2], F32, tag="mv")
                    nc.vector.bn_aggr(out=mv, in_=stats)
                    sq = ln_small.tile([128, 1], F32, tag="sq")
                    nc.scalar.activation(
                        out=sq, in_=mv[:, 1:2], func=FT.Sqrt, bias=ln_eps
                    )
                    rstd = ln_small.tile([128, 1], F32, tag="rstd")
                    nc.vector.reciprocal(out=rstd, in_=sq)
                    nmr = ln_small.tile([128, 1], F32, tag="nmr")
                    nc.vector.tensor_scalar(
                        out=nmr,
                        in0=mv[:, 0:1],
                        scalar1=rstd,
                        scalar2=-1.0,
                        op0=ALU.mult,
                        op1=ALU.mult,
                    )
                    ln_ctx[qt_i] = (rstd, nmr)

                def emit_ln1_apply(qt_i):
                    rstd, nmr = ln_ctx.pop(qt_i)
                    xn = lnpool.tile([128, D], F32, tag="lnbig")
                    nc.scalar.activation(
                        out=xn,
                        in_=x_sb[:, qt_i, :],
                        func=FT.Identity,
                        bias=nmr,
                        scale=rstd,
                    )
                    xg = lnpool.tile([128, D], F32, tag="lnbig2")
                    nc.vector.tensor_mul(out=xg, in0=xn, in1=g1b)
                    nc.vector.tensor_add(out=x_sb[:, qt_i, :], in0=xg, in1=be1b)

                def emit_transposes(qt_i):
                    for ctg in range(2):
                        pt = ps_b.tile([128, 512], F32R, tag="psb")
                        for j in range(4):
                            ct = ctg * 4 + j
                            nc.tensor.transpose(
                                pt[:, j * 128 : j * 128 + 128],
                                x_sb[:, qt_i, ct * 128 : ct * 128 + 128],
                                ident,
                            )
                        nc.vector.tensor_copy(
                            out=xT[
                                :, ctg * 4 : ctg * 4 + 4, qt_i * 128 : qt_i * 128 + 128
                            ],
                            in_=pt.rearrange("p (c k) -> p c k", c=4),
                        )

                # out-proj (qt0/qt1 mostly pre-computed in the attention
                # tail), LN1 software-pipelined: each block's Act work is
                # emitted before the NEXT block's stats so the in-order Act
                # queue never parks xn behind a later sqrt.
                emit_outproj(0)
                emit_ln1_stats(0)
                emit_outproj(1)
                emit_ln1_stats(1)
                emit_outproj(2)
                emit_ln1_apply(0)
                emit_outproj(3)
                emit_ln1_apply(1)
                emit_transposes(0)
                emit_ln1_stats(2)
                emit_ln1_apply(2)
                emit_transposes(1)
                emit_ln1_stats(3)
                emit_ln1_apply(3)
                emit_transposes(2)
                emit_transposes(3)

                # ---- FFN mm1 + relu: h[f, q] bf16 ----
                h_sb = hpool.tile([128, 32, 512], BF16, tag="h_sb")
                for fc in range(16):
                    wb = w1p.tile([128, 8, 256], BF16, tag="w1")
                    nc.sync.dma_start(out=wb, in_=W1[fc, :, :, :])
                    for fl in range(2):
                        ft = fc * 2 + fl
                        psum = ps_b.tile([128, 512], F32, tag="psb")
                        for co in range(8):
                            nc.tensor.matmul(
                                psum,
                                wb[:, co, fl * 128 : fl * 128 + 128],
                                xT[:, co, :],
                                start=(co == 0),
                                stop=(co == 7),
                            )
                        nc.scalar.activation(
                            out=h_sb[:, ft, :],
                            in_=psum,
                            func=FT.Relu,
                            bias=b12[:, ft : ft + 1],
                        )

                # ---- FFN mm2 (4 column-quarter passes) + LN2 + out ----
                def emit_ln2_out(qt_i):
                    yp = ypre[:, qt_i, :]
                    stats = ln_small.tile([128, 2, 6], F32, tag="stats")
                    for half in range(2):
                        nc.vector.bn_stats(
                            out=stats[:, half, :],
                            in_=yp[:, half * 512 : half * 512 + 512],
                        )
                    mv = ln_small.tile([128, 2], F32, tag="mv")
                    nc.vector.bn_aggr(out=mv, in_=stats)
                    sq = ln_small.tile([128, 1], F32, tag="sq")
                    nc.scalar.activation(
                        out=sq, in_=mv[:, 1:2], func=FT.Sqrt, bias=ln_eps
                    )
                    rstd = ln_small.tile([128, 1], F32, tag="rstd")
                    nc.vector.reciprocal(out=rstd, in_=sq)
                    nmr = ln_small.tile([128, 1], F32, tag="nmr")
                    nc.vector.tensor_scalar(
                        out=nmr,
                        in0=mv[:, 0:1],
                        scalar1=rstd,
                        scalar2=-1.0,
                        op0=ALU.mult,
                        op1=ALU.mult,
                    )
                    yn = lnpool.tile([128, D], F32, tag="lnbig")
                    nc.scalar.activation(
                        out=yn, in_=yp, func=FT.Identity, bias=nmr, scale=rstd
                    )
                    yg = lnpool.tile([128, D], F32, tag="lnbig2")
                    nc.vector.tensor_mul(out=yg, in0=yn, in1=g2b)
                    out_t = lnpool.tile([128, D], F32, tag="lnbig")
                    nc.vector.tensor_add(out=out_t, in0=yg, in1=be2b)
                    nc.sync.dma_start(
                        out=Out[qt_i * 128 : qt_i * 128 + 128, :], in_=out_t
                    )

                for pq in range(4):
                    if pq >= 2:
                        w2_tiles[pq] = w2p.tile(
                            [128, 32, 256], BF16, tag="w2", name=f"w2t{pq}"
                        )
                        nc.scalar.dma_start(out=w2_tiles[pq], in_=W2[pq, :, :, :])
                    w2t = w2_tiles[pq]
                    for qt_i in range(4):
                        psum = ps_b.tile([128, 512], F32, tag="psb")
                        nc.tensor.matmul(
                            psum[:, 0:256],
                            ones_row[0:1, 0:128],
                            b2_r[0:1, pq * 256 : pq * 256 + 256],
                            start=True,
                            stop=False,
                        )
                        for ft in range(32):
                            nc.tensor.matmul(
                                psum[:, 0:256],
                                h_sb[:, ft, qt_i * 128 : qt_i * 128 + 128],
                                w2t[:, ft, :],
                                start=False,
                                stop=(ft == 31),
                            )
                        nc.vector.tensor_add(
                            out=ypre[:, qt_i, pq * 256 : pq * 256 + 256],
                            in0=psum[:, 0:256],
                            in1=x_sb[:, qt_i, pq * 256 : pq * 256 + 256],
                        )
                        if pq == 3:
                            emit_ln2_out(qt_i)

    _legalize_waits(nc)
    return nc


_CACHE = {}


def kernel(**inputs):
    import os

    if "nc" not in _CACHE:
        _CACHE["nc"] = _build()
    nc = _CACHE["nc"]

    f32 = np.float32
    bf16 = ml_dtypes.bfloat16

    def relay(w, ki=128):
        """[N*ki, M] -> [ki, N, M] contiguous bf16."""
        n = w.shape[0] // ki
        return np.ascontiguousarray(
            w.reshape(n, ki, w.shape[1]).transpose(1, 0, 2).astype(bf16)
        )

    src = np.asarray(inputs["src"], f32)
    distances = np.asarray(inputs["distances"], f32)
    scale = np.float32(HD ** -0.5)
    Wq_s = np.asarray(inputs["Wq"], f32) * scale
    bq_s = np.asarray(inputs["bq"], f32) * scale
    Wk_f = np.asarray(inputs["Wk"], f32)
    Wv_f = np.asarray(inputs["Wv"], f32)
    Wo_f = np.asarray(inputs["Wo"], f32)
    W1_f = np.asarray(inputs["W1"], f32)
    W2_f = np.asarray(inputs["W2"], f32)

    # chunked relayouts: [ki, ko, cols]
    wk_r = np.stack([relay(Wk_f[:, wc * 256 : wc * 256 + 256]) for wc in range(4)])
    wq_r = np.stack([relay(Wq_s[:, wc * 256 : wc * 256 + 256]) for wc in range(4)])
    wv_r = np.stack([relay(Wv_f[:, vc * 512 : vc * 512 + 512]) for vc in range(2)])
    wo_r = relay(Wo_f)
    w1_r = np.stack([relay(W1_f[:, fc * 256 : fc * 256 + 256]) for fc in range(16)])
    w2_r = np.stack([relay(W2_f[:, pq * 256 : pq * 256 + 256]) for pq in range(4)])

    c3 = np.zeros((128, 48), f32)
    c3[:, 0:8] = bq_s.reshape(8, 128).T
    c3[:, 8:16] = np.asarray(inputs["bk"], f32).reshape(8, 128).T
    c3[:, 16:48] = np.asarray(inputs["b1"], f32).reshape(32, 128).T

    rep = lambda v: np.ascontiguousarray(
        np.broadcast_to(np.asarray(v, f32).astype(bf16)[None, :], (128, D))
    )

    shared = {
        "wk": wk_r, "wq": wq_r, "wv": wv_r, "wo": wo_r, "w1": w1_r, "w2": w2_r,
        "c3": c3,
        "bv_r": np.asarray(inputs["bv"], f32).reshape(1, D).copy(),
        "b2_r": np.asarray(inputs["b2"], f32).reshape(1, D).copy(),
        "g1b": rep(inputs["g1"]),
        "beta1b": rep(inputs["beta1"]),
        "g2b": rep(inputs["g2"]),
        "beta2b": rep(inputs["beta2"]),
        "ident": np.eye(128, dtype=f32),
        "ones_row": np.ones((1, 512), f32),
    }

    negabs = -abs(float(np.asarray(inputs["dist_scale"])))
    bo = np.asarray(inputs["bo"], f32)

    in_maps = []
    for c in range(NCORES):
        b, qh = c // 2, c % 2
        q0 = qh * SQ
        if qh == 0:
            perm = np.arange(S)
        else:
            perm = np.r_[np.arange(512, 1024), np.arange(0, 512)]
        m = dict(shared)
        srcTb = src[b][perm].T.astype(bf16)                       # [d, s]
        m["srcT"] = np.ascontiguousarray(
            srcTb.reshape(8, 128, S).transpose(1, 0, 2)
        )
        sq = (src[b, q0 : q0 + SQ] + bo[None, :]).astype(f32)     # [q, d]
        m["src_q"] = np.ascontiguousarray(
            sq.reshape(4, 128, D).transpose(1, 0, 2)
        )
        dT = distances[b, q0 : q0 + SQ][:, perm].T                # [k, q]
        p = np.exp(negabs * np.log(dT + 1e-9)).astype(bf16)
        m["pt"] = np.ascontiguousarray(p.reshape(8, 128, SQ).transpose(1, 0, 2))
        in_maps.append(m)

    trace = bool(int(os.environ.get("BASS_KERNEL_TRACE", "0")))
    res = run_bass_kernel_spmd(
        nc,
        in_maps,
        core_ids=list(range(NCORES)),
        trace=trace,
        stitch_traces=False,
    )
    _CACHE["last_result"] = res

    out = np.empty((B, S, D), f32)
    for c in range(NCORES):
        b, qh = c // 2, c % 2
        out[b, qh * SQ : qh * SQ + SQ] = res.results[c]["out"]
    return out


# revision 23
# speedup vs baseline: 1.2379x; 1.2379x over previous
"""Distance-aware transformer encoder layer on 8 Trainium2 NeuronCores.

Sharding: core c handles batch b = c//2 and query-half qh = c%2 (512 query
rows). K/V are computed per-core for the full 1024-key sequence of its batch
(duplicated across the core pair — cheaper than a collective). Everything
else (scores, softmax, out-proj, LayerNorms, FFN) is perfectly sharded by
query rows. No collectives.

v2 changes vs baseline (653us):
- distance bias folded multiplicatively: E = exp(scores) * P with
  P = (d+1e-9)^(-|dist_scale|) precomputed HOST-side and DMA'd in bf16.
  Removes the 65K-row ident@biasT PE matmuls and the on-chip ln/exp chain
  that serialized the startup DMA queue.
- all weights shipped bf16 and host-relaid so every DMA is 128 contiguous
  per-partition runs (no descriptor storms).
- two HW DMA queues: SP carries the critical stream (srcT, Wk/Wq/Wv, P,
  src_q, W1, out), Act carries never-blocking prefetches (LN params, Wo, W2).
- attention pipelined per-head (scores / exp / P-mult / attnV overlap), with
  the second half of the QKV projections interleaved between heads to keep
  the PE dense while Act does exp.
- softmax denominator reciprocal via DVE reciprocal_approx_fast (the exact
  `reciprocal` cost 2.3us/head) broadcast across partitions with a K=1
  ones-outer-product matmul instead of a DRAM round-trip.
- LayerNorm rstd via single Rsqrt activation.
- FFN2 streams W2 in 4 column-quarter tiles (bufs=2) so LN2 + output DMA
  pipeline behind the last pass.
"""

import numpy as np
import ml_dtypes

import bass_rust
import concourse.bass as bass
import concourse.tile as tile
import concourse.mybir as mybir
from concourse.bass import AP
from concourse.bass_utils import run_bass_kernel_spmd

B, S, D, H, DFF, HD = 4, 1024, 1024, 16, 4096, 64
SQ = 512          # query rows per core
NCORES = 8
EPS = 1e-5
F32 = mybir.dt.float32
F32R = mybir.dt.float32r
BF16 = mybir.dt.bfloat16
FT = mybir.ActivationFunctionType
ALU = mybir.AluOpType

SPIN = 52

_nop_ctr = [0]


def _legalize_waits(nc):
    """walrus codegen in this toolchain accepts only one sync-wait per
    instruction; split extras onto same-engine NoOps inserted before."""
    n_fixed = 0
    for f in nc.m.functions:
        for bb in f.blocks:
            insts = bb.instructions
            i = 0
            while i < len(insts):
                inst = insts[i]
                si = inst.sync_info
                waits = list(si.on_wait) if si is not None and si.on_wait else []
                if len(waits) > 1:
                    keep = waits[-1]
                    for w in waits[:-1]:
                        n = bass_rust.InstNoOp(
                            name=f"waitsplit-nop-{_nop_ctr[0]}", ins=[], outs=[]
                        )
                        _nop_ctr[0] += 1
                        n.engine = inst.engine
                        n.sync_info = bass_rust.SyncInfo(on_update=[], on_wait=[w])
                        insts.insert(i, n)
                        i += 1
                    inst.sync_info = bass_rust.SyncInfo(
                        on_update=list(si.on_update or []), on_wait=[keep]
                    )
                    n_fixed += 1
                i += 1
    return n_fixed


def _build():
    nc = bass.Bass()
    dp = nc.declare_dram_parameter

    SrcT = dp("srcT", [128, 8, S], BF16, isOutput=False)    # [ki, ko, s]
    SrcQ = dp("src_q", [128, 4, D], F32, isOutput=False)    # [ki, qt, d] (+bo)
    PT = dp("pt", [128, 8, SQ], BF16, isOutput=False)       # P[k,q] [ki,ko,q]
    Wk = dp("wk", [4, 128, 8, 256], BF16, isOutput=False)   # [wc][ki][ko][dout]
    Wq = dp("wq", [4, 128, 8, 256], BF16, isOutput=False)   # pre-scaled HD^-.5
    Wv = dp("wv", [2, 128, 8, 512], BF16, isOutput=False)   # [vc2][ki][ko][dout]
    Wo = dp("wo", [128, 8, D], BF16, isOutput=False)        # [ki][dp][dout]
    W1 = dp("w1", [16, 128, 8, 256], BF16, isOutput=False)  # [fc][ki][co][f]
    W2 = dp("w2", [4, 128, 32, 256], BF16, isOutput=False)  # [pq][fi][fo][d]
    C3 = dp("c3", [128, 48], F32, isOutput=False)           # bq2|bk2|b12
    BvR = dp("bv_r", [1, D], F32R, isOutput=False)
    B2R = dp("b2_r", [1, D], F32R, isOutput=False)
    G1 = dp("g1b", [128, D], BF16, isOutput=False)
    G2 = dp("g2b", [128, D], BF16, isOutput=False)
    Be2 = dp("beta2b", [128, D], BF16, isOutput=False)
    Ident = dp("ident", [128, 128], F32R, isOutput=False)
    OnesR = dp("ones_row", [1, 512], F32R, isOutput=False)
    Out = dp("out", [SQ, D], F32, isOutput=True)

    with tile.TileContext(nc) as tc:
        import contextlib

        ctx = contextlib.ExitStack()
        with ctx:
            consts = ctx.enter_context(tc.tile_pool(name="consts", bufs=1))
            ao_pool = ctx.enter_context(tc.tile_pool(name="ao_pool", bufs=1))
            wo_pool = ctx.enter_context(tc.tile_pool(name="wo_pool", bufs=1))
            srcq_pool = ctx.enter_context(tc.tile_pool(name="srcq_pool", bufs=1))
            small = ctx.enter_context(tc.tile_pool(name="small", bufs=2))
            ln_small = ctx.enter_context(tc.tile_pool(name="ln_small", bufs=4))
            rsb_pool = ctx.enter_context(tc.tile_pool(name="rsb", bufs=2))
            stash_pool = ctx.enter_context(tc.tile_pool(name="stash", bufs=4))
            ps_a = ctx.enter_context(tc.tile_pool(name="ps_a", bufs=2, space="PSUM"))
            ps_b = ctx.enter_context(tc.tile_pool(name="ps_b", bufs=4, space="PSUM"))

            # ---- consts: only what the first K/Q matmuls need goes first ----
            c3 = consts.tile([128, 48], F32, tag="c3")
            nc.sync.dma_start(out=c3, in_=C3[:, :])
            bq2 = c3[:, 0:8]
            bk2 = c3[:, 8:16]
            b12 = c3[:, 16:48]
            ident = consts.tile([128, 128], F32R, tag="ident")
            ones_row = consts.tile([1, 512], F32R, tag="ones_row")
            bv_r = consts.tile([1, D], F32R, tag="bv_r")
            b2_r = consts.tile([1, D], F32R, tag="b2_r")

            # ---- prefetches on Act queue (never block) ----
            g1b = consts.tile([128, D], BF16, tag="g1b")
            nc.scalar.dma_start(out=g1b, in_=G1[:, :])
            g2b = consts.tile([128, D], BF16, tag="g2b")
            nc.scalar.dma_start(out=g2b, in_=G2[:, :])
            be2b = consts.tile([128, D], BF16, tag="be2b")
            nc.scalar.dma_start(out=be2b, in_=Be2[:, :])
            wo_sb = wo_pool.tile([128, 8, D], BF16, tag="wo")
            nc.scalar.dma_start(out=wo_sb, in_=Wo[:, :, :])

            ln_eps = consts.tile([128, 1], F32, tag="ln_eps")
            nc.vector.memset(ln_eps, EPS)
            spin_src = consts.tile([128, 512], BF16, tag="spin_src")
            nc.vector.memset(spin_src, 0.0)

            # ---- spin: ramp the PE while the first DMAs land ----
            for _ in range(SPIN):
                sp = ps_b.tile([128, 512], F32, tag="psb")
                nc.tensor.matmul(sp, spin_src[:, 0:128], spin_src, start=True, stop=True)

            # ---- persistent activations ----
            ao_sb = ao_pool.tile([128, 8, 512], BF16, tag="ao")
            src_q = srcq_pool.tile([128, 4, D], F32, tag="srcq")

            with tc.tile_pool(name="attn", bufs=1) as attn_pool, \
                 tc.tile_pool(name="wkq", bufs=2) as wkq_pool, \
                 tc.tile_pool(name="wv2", bufs=2) as wv_pool, \
                 tc.tile_pool(name="epool", bufs=6) as epool:

                srcT = attn_pool.tile([128, 8, S], BF16, tag="srcT")
                nc.sync.dma_start(out=srcT[:, 0:4, :], in_=SrcT[:, 0:4, :])
                nc.scalar.dma_start(out=srcT[:, 4:8, :], in_=SrcT[:, 4:8, :])
                kt = attn_pool.tile([128, 8, S], BF16, tag="kt")
                qt = attn_pool.tile([128, 8, SQ], BF16, tag="qt")
                p_sb = attn_pool.tile([128, 8, SQ], BF16, tag="p_sb")
                v_sb = attn_pool.tile([128, 8, 16, 65], BF16, tag="v_sb")
                nc.vector.memset(v_sb[:, :, :, 64:65], 1.0)

                _kbufs, _qbufs = {}, {}

                def emit_k_chunk(wc, groups=None):
                    if wc not in _kbufs:
                        wb = wkq_pool.tile(
                            [128, 8, 256], BF16, tag="wkq", name=f"wkb{wc}"
                        )
                        _kbufs[wc] = wb
                        nc.sync.dma_start(out=wb, in_=Wk[wc, :, :, :])
                    wb = _kbufs[wc]
                    for g in range(4) if groups is None else groups:
                        dl, nt = g // 2, g % 2
                        dt = wc * 2 + dl
                        psum = ps_b.tile([128, 512], F32, tag="psb")
                        for ko in range(8):
                            nc.tensor.matmul(
                                psum,
                                wb[:, ko, dl * 128 : dl * 128 + 128],
                                srcT[:, ko, nt * 512 : nt * 512 + 512],
                                start=(ko == 0),
                                stop=(ko == 7),
                            )
                        nc.vector.tensor_scalar_add(
                            out=kt[:, dt, nt * 512 : nt * 512 + 512],
                            in0=psum,
                            scalar1=bk2[:, dt : dt + 1],
                        )

                def emit_q_chunk(wc, groups=None):
                    if wc not in _qbufs:
                        wb = wkq_pool.tile(
                            [128, 8, 256], BF16, tag="wkq", name=f"wqb{wc}"
                        )
                        _qbufs[wc] = wb
                        nc.sync.dma_start(out=wb, in_=Wq[wc, :, :, :])
                    wb = _qbufs[wc]
                    for dl in range(2) if groups is None else groups:
                        dt = wc * 2 + dl
                        psum = ps_b.tile([128, 512], F32, tag="psb")
                        for ko in range(8):
                            nc.tensor.matmul(
                                psum,
                                wb[:, ko, dl * 128 : dl * 128 + 128],
                                srcT[:, ko, 0:512],
                                start=(ko == 0),
                                stop=(ko == 7),
                            )
                        nc.vector.tensor_scalar_add(
                            out=qt[:, dt, :], in0=psum, scalar1=bq2[:, dt : dt + 1]
                        )

                def emit_v_chunk(vc2, mts, load):
                    if load:
                        wb = wv_pool.tile(
                            [128, 8, 512], BF16, tag="wv", name=f"wvb{vc2}"
                        )
                        emit_v_chunk.wb[vc2] = wb
                        nc.sync.dma_start(out=wb, in_=Wv[vc2, :, :, :])
                    wb = emit_v_chunk.wb[vc2]
                    for mt in mts:
                        psum = ps_b.tile([128, 512], F32, tag="psb")
                        nc.tensor.matmul(
                            psum,
                            ones_row[0:1, 0:128],
                            bv_r[0:1, vc2 * 512 : vc2 * 512 + 512],
                            start=True,
                            stop=False,
                        )
                        for ko in range(8):
                            nc.tensor.matmul(
                                psum,
                                srcT[:, ko, mt * 128 : mt * 128 + 128],
                                wb[:, ko, :],
                                start=False,
                                stop=(ko == 7),
                            )
                        nc.vector.tensor_copy(
                            out=v_sb[:, mt, vc2 * 8 : vc2 * 8 + 8, 0:64],
                            in_=psum.rearrange("p (h e) -> p h e", e=64),
                        )

                emit_v_chunk.wb = {}

                # ---- group 0: everything heads 0-7 need ----
                emit_k_chunk(0)
                emit_q_chunk(0)
                emit_k_chunk(1)
                emit_q_chunk(1)
                # V-bias consts must beat the group-0 V matmuls; they are tiny
                nc.sync.dma_start(out=ones_row, in_=OnesR[:, :])
                nc.sync.dma_start(out=bv_r, in_=BvR[:, :])
                nc.sync.dma_start(out=b2_r, in_=B2R[:, :])
                nc.sync.dma_start(out=ident, in_=Ident[:, :])
                emit_v_chunk(0, range(8), load=True)
                # P + src_q loads behind group-0 weights on SP
                nc.sync.dma_start(out=p_sb, in_=PT[:, :, :])
                nc.sync.dma_start(out=src_q, in_=SrcQ[:, :, :])

                # group-1 work interleaved between attention heads at
                # single-psum-group granularity (bounded PSUM footprint)
                def k_units(wc):
                    units = []
                    for i in range(4):
                        units.append(lambda wc=wc, i=i: emit_k_chunk(wc, [i]))
                    return units

                def q_units(wc):
                    return [
                        lambda wc=wc: emit_q_chunk(wc, [0]),
                        lambda wc=wc: emit_q_chunk(wc, [1]),
                    ]

                g1_units = (
                    k_units(2)
                    + q_units(2)
                    + [lambda mt=mt: emit_v_chunk(1, [mt], load=(mt == 0))
                       for mt in range(8)]
                    + k_units(3)
                    + q_units(3)
                )
                # units per head-iteration; deadlines: K2/Q2 by iter 7,
                # V1 by iter 8, K3/Q3 by iter 11. PSUM allows 2/iter
                # (4 at iter 0 where no pao/pr tiles are live).
                g1_sched = [2, 2, 2, 2, 2, 1, 1, 1, 1, 2, 2, 2, 0, 0, 0, 0]

                # ---- attention, per-head pipeline ----
                e_tiles = {}
                rec_r = {}
                pao = {}

                def emit_scores_kog(h, kog):
                    base, dt = (h % 2) * 64, h // 2
                    pss = ps_a.tile([128, 2, 512], F32, tag="pss")
                    for kl in range(2):
                        ko = kog * 2 + kl
                        nc.tensor.matmul(
                            pss[:, kl, :],
                            kt[base : base + 64, dt, ko * 128 : ko * 128 + 128],
                            qt[base : base + 64, dt, :],
                            start=True,
                            stop=True,
                        )
                    e_t = epool.tile([128, 2, 512], BF16, tag="e_t")
                    nc.scalar.activation(out=e_t, in_=pss, func=FT.Exp)
                    nc.vector.tensor_mul(
                        out=e_t, in0=e_t, in1=p_sb[:, kog * 2 : kog * 2 + 2, :]
                    )
                    e_tiles[(h, kog)] = e_t

                def emit_attnv(h):
                    pa = ps_b.tile([128, 512], F32, tag="psb")
                    pao[h] = pa
                    for ko in range(8):
                        nc.tensor.matmul(
                            pa[0:65, :],
                            v_sb[:, ko, h, :],
                            e_tiles[(h, ko // 2)][:, ko % 2, :],
                            start=(ko == 0),
                            stop=(ko == 7),
                        )
                    for kog in range(4):
                        del e_tiles[(h, kog)]

                def emit_rec(h):
                    # 1/denom as Exp(-Ln(denom)) on Act: avoids the 2.3us
                    # exact DVE reciprocal and custom-ISA approx ops. For the
                    # last heads (no proj filler; Act is the bottleneck) use
                    # the exact DVE reciprocal instead to rebalance engines.
                    rr = small.tile([1, 512], F32R, tag="rec_r")
                    with nc.allow_low_precision(reason="f32r is f32-bit"):
                        nc.scalar.activation(
                            out=rr, in_=pao[h][64:65, :], func=FT.Ln
                        )
                        nc.scalar.activation(
                            out=rr, in_=rr, func=FT.Exp, scale=-1.0
                        )
                    rec_r[h] = rr

                def emit_norm(h):
                    base, dt = (h % 2) * 64, h // 2
                    pr = ps_b.tile([128, 512], F32, tag="psb")
                    nc.tensor.matmul(
                        pr[0:64, :], ones_row[0:1, 0:64], rec_r[h][0:1, :],
                        start=True, stop=True,
                    )
                    rec_sb = rsb_pool.tile([64, 512], F32, tag="rec_sb")
                    nc.vector.tensor_copy(out=rec_sb, in_=pr[0:64, :])
                    nc.vector.tensor_mul(
                        out=ao_sb[base : base + 64, dt, :],
                        in0=pao[h][0:64, :],
                        in1=rec_sb,
                    )
                    del pao[h], rec_r[h]

                # partial out-projection (heads 0-11 contribution) emitted
                # into the attention tail, where Act/DVE otherwise rate-limit
                # the PE. Results stash to SBUF; dpi 6-7 finish after norms.
                op_stash = {}

                def emit_op_part(qt_i, nt):
                    psum = ps_b.tile(
                        [128, 512], F32, tag="psb", name=f"oppp{qt_i}{nt}"
                    )
                    for j, dpi in enumerate(range(6)):
                        nc.tensor.matmul(
                            psum,
                            ao_sb[:, dpi, qt_i * 128 : qt_i * 128 + 128],
                            wo_sb[:, dpi, nt * 512 : nt * 512 + 512],
                            start=(j == 0),
                            stop=(j == 5),
                        )
                    st = stash_pool.tile(
                        [128, 512], BF16, tag="opst", name=f"opst{qt_i}{nt}"
                    )
                    nc.vector.tensor_copy(out=st, in_=psum)
                    op_stash[(qt_i, nt)] = st

                ui = 0
                for h in range(H):
                    emit_scores_kog(h, 0)
                    emit_scores_kog(h, 1)
                    if h >= 1:
                        emit_attnv(h - 1)
                        emit_rec(h - 1)
                    emit_scores_kog(h, 2)
                    emit_scores_kog(h, 3)
                    n_units = g1_sched[h] if h > 0 else 4
                    for _ in range(n_units):
                        if ui < len(g1_units):
                            g1_units[ui]()
                            ui += 1
                    if h >= 1:
                        emit_norm(h - 1)
                    if h == 13:
                        emit_op_part(0, 0)
                    elif h == 14:
                        emit_op_part(0, 1)
                    elif h == 15:
                        emit_op_part(1, 0)
                emit_attnv(15)
                emit_op_part(1, 1)
                emit_rec(15)
                emit_norm(15)

            # ================= scope 2: out-proj / LN1 / FFN =================
            with tc.tile_pool(name="xpool", bufs=1) as xpool, \
                 tc.tile_pool(name="hpool", bufs=1) as hpool, \
                 tc.tile_pool(name="w1p", bufs=3) as w1p, \
                 tc.tile_pool(name="w2p", bufs=2) as w2p, \
                 tc.tile_pool(name="lnpool", bufs=2) as lnpool:

                x_sb = xpool.tile([128, 4, D], F32R, tag="x")      # xpre then x
                ypre = xpool.tile([128, 4, D], F32, tag="ypre")
                xT = xpool.tile([128, 8, 512], BF16, tag="xT")

                # W2 first two column-quarters prefetch on the Act queue now,
                # so the transfers overlap out-proj/LN1/FFN1.
                w2_tiles = {}
                for pq in range(2):
                    w2_tiles[pq] = w2p.tile(
                        [128, 32, 256], BF16, tag="w2", name=f"w2t{pq}"
                    )
                    nc.scalar.dma_start(out=w2_tiles[pq], in_=W2[pq, :, :, :])

                def emit_outproj(qt_i):
                    for nt in range(2):
                        st = op_stash.pop((qt_i, nt), None)
                        psum = ps_b.tile([128, 512], F32, tag="psb")
                        dpis = range(6, 8) if st is not None else range(8)
                        for j, dpi in enumerate(dpis):
                            nc.tensor.matmul(
                                psum,
                                ao_sb[:, dpi, qt_i * 128 : qt_i * 128 + 128],
                                wo_sb[:, dpi, nt * 512 : nt * 512 + 512],
                                start=(j == 0),
                                stop=(dpi == 7),
                            )
                        xs = x_sb[:, qt_i, nt * 512 : nt * 512 + 512]
                        if st is not None:
                            nc.vector.tensor_add(out=xs, in0=psum, in1=st)
                            nc.vector.tensor_add(
                                out=xs,
                                in0=xs,
                                in1=src_q[:, qt_i, nt * 512 : nt * 512 + 512],
                            )
                        else:
                            nc.vector.tensor_add(
                                out=xs,
                                in0=psum,
                                in1=src_q[:, qt_i, nt * 512 : nt * 512 + 512],
                            )

                ln_ctx = {}

                def emit_ln1_stats(qt_i):
                    xpre = x_sb[:, qt_i, :]
                    stats = ln_small.tile([128, 2, 6], F32, tag="stats")
                    for half in range(2):
                        nc.vector.bn_stats(
                            out=stats[:, half, :],
                            in_=xpre[:, half * 512 : half * 512 + 512],
                        )
                    mv = ln_small.tile([128, 2], F32, tag="mv")
                    nc.vector.bn_aggr(out=mv, in_=stats)
                    sq = ln_small.tile([128, 1], F32, tag="sq")
                    nc.scalar.activation(
                        out=sq, in_=mv[:, 1:2], func=FT.Sqrt, bias=ln_eps
                    )
                    rstd = ln_small.tile([128, 1], F32, tag="rstd")
                    nc.vector.reciprocal(out=rstd, in_=sq)
                    nmr = ln_small.tile([128, 1], F32, tag="nmr")
                    nc.vector.tensor_scalar(
                        out=nmr,
                        in0=mv[:, 0:1],
                        scalar1=rstd,
                        scalar2=-1.0,
                        op0=ALU.mult,
                        op1=ALU.mult,
                    )
                    ln_ctx[qt_i] = (rstd, nmr)

                def emit_ln1_apply(qt_i):
                    # writes x-hat (normalized, un-affined) in place; gamma is
                    # folded into W1 host-side and beta1 into b2/b1, so only
                    # the FFN2-residual path needs g1*x-hat — computed on the
                    # otherwise-idle GpSimd engine into the ypre accumulator.
                    rstd, nmr = ln_ctx.pop(qt_i)
                    with nc.allow_low_precision(reason="f32r is f32-bit"):
                        nc.scalar.activation(
                            out=x_sb[:, qt_i, :],
                            in_=x_sb[:, qt_i, :],
                            func=FT.Identity,
                            bias=nmr,
                            scale=rstd,
                        )
                    nc.gpsimd.tensor_mul(
                        out=ypre[:, qt_i, :], in0=x_sb[:, qt_i, :], in1=g1b
                    )

                def emit_transposes(qt_i):
                    for ctg in range(2):
                        pt = ps_b.tile([128, 512], F32R, tag="psb")
                        for j in range(4):
                            ct = ctg * 4 + j
                            nc.tensor.transpose(
                                pt[:, j * 128 : j * 128 + 128],
                                x_sb[:, qt_i, ct * 128 : ct * 128 + 128],
                                ident,
                            )
                        nc.scalar.activation(
                            out=xT[
                                :, ctg * 4 : ctg * 4 + 4, qt_i * 128 : qt_i * 128 + 128
                            ],
                            in_=pt.rearrange("p (c k) -> p c k", c=4),
                            func=FT.Identity,
                        )

                # out-proj (qt0/qt1 mostly pre-computed in the attention
                # tail), then LN1 software-pipelined across row blocks
                emit_outproj(0)
                emit_outproj(1)
                emit_outproj(2)
                emit_ln1_stats(0)
                emit_ln1_stats(1)
                emit_outproj(3)
                emit_ln1_stats(2)
                emit_ln1_stats(3)
                emit_ln1_apply(0)
                emit_ln1_apply(1)
                emit_transposes(0)
                emit_ln1_apply(2)
                emit_transposes(1)
                emit_ln1_apply(3)
                emit_transposes(2)
                emit_transposes(3)

                # ---- FFN mm1 + relu: h[f, q] bf16 ----
                h_sb = hpool.tile([128, 32, 512], BF16, tag="h_sb")
                for fc in range(16):
                    wb = w1p.tile([128, 8, 256], BF16, tag="w1")
                    nc.sync.dma_start(out=wb, in_=W1[fc, :, :, :])
                    for fl in range(2):
                        ft = fc * 2 + fl
                        psum = ps_b.tile([128, 512], F32, tag="psb")
                        for co in range(8):
                            nc.tensor.matmul(
                                psum,
                                wb[:, co, fl * 128 : fl * 128 + 128],
                                xT[:, co, :],
                                start=(co == 0),
                                stop=(co == 7),
                            )
                        nc.scalar.activation(
                            out=h_sb[:, ft, :],
                            in_=psum,
                            func=FT.Relu,
                            bias=b12[:, ft : ft + 1],
                        )

                # ---- FFN mm2 (4 column-quarter passes) + LN2 + out ----
                def emit_ln2_out(qt_i):
                    yp = ypre[:, qt_i, :]
                    stats = ln_small.tile([128, 2, 6], F32, tag="stats")
                    for half in range(2):
                        nc.vector.bn_stats(
                            out=stats[:, half, :],
                            in_=yp[:, half * 512 : half * 512 + 512],
                        )
                    mv = ln_small.tile([128, 2], F32, tag="mv")
                    nc.vector.bn_aggr(out=mv, in_=stats)
                    sq = ln_small.tile([128, 1], F32, tag="sq")
                    nc.scalar.activation(
                        out=sq, in_=mv[:, 1:2], func=FT.Sqrt, bias=ln_eps
                    )
                    rstd = ln_small.tile([128, 1], F32, tag="rstd")
                    nc.vector.reciprocal(out=rstd, in_=sq)
                    nmr = ln_small.tile([128, 1], F32, tag="nmr")
                    nc.vector.tensor_scalar(
                        out=nmr,
                        in0=mv[:, 0:1],
                        scalar1=rstd,
                        scalar2=-1.0,
                        op0=ALU.mult,
                        op1=ALU.mult,
                    )
                    yn = lnpool.tile([128, D], F32, tag="lnbig")
                    nc.scalar.activation(
                        out=yn, in_=yp, func=FT.Identity, bias=nmr, scale=rstd
                    )
                    yg = lnpool.tile([128, D], F32, tag="lnbig2")
                    eng = nc.vector if qt_i == 3 else nc.gpsimd
                    eng.tensor_mul(out=yg, in0=yn, in1=g2b)
                    out_t = lnpool.tile([128, D], F32, tag="lnbig")
                    eng.tensor_add(out=out_t, in0=yg, in1=be2b)
                    nc.sync.dma_start(
                        out=Out[qt_i * 128 : qt_i * 128 + 128, :], in_=out_t
                    )

                for pq in range(4):
                    if pq >= 2:
                        w2_tiles[pq] = w2p.tile(
                            [128, 32, 256], BF16, tag="w2", name=f"w2t{pq}"
                        )
                        nc.scalar.dma_start(out=w2_tiles[pq], in_=W2[pq, :, :, :])
                    w2t = w2_tiles[pq]
                    for qt_i in range(4):
                        psum = ps_b.tile([128, 512], F32, tag="psb")
                        nc.tensor.matmul(
                            psum[:, 0:256],
                            ones_row[0:1, 0:128],
                            b2_r[0:1, pq * 256 : pq * 256 + 256],
                            start=True,
                            stop=False,
                        )
                        for ft in range(32):
                            nc.tensor.matmul(
                                psum[:, 0:256],
                                h_sb[:, ft, qt_i * 128 : qt_i * 128 + 128],
                                w2t[:, ft, :],
                                start=False,
                                stop=(ft == 31),
                            )
                        nc.vector.tensor_add(
                            out=ypre[:, qt_i, pq * 256 : pq * 256 + 256],
                            in0=psum[:, 0:256],
                            in1=ypre[:, qt_i, pq * 256 : pq * 256 + 256],
                        )
                        if pq == 3:
                            emit_ln2_out(qt_i)

    _legalize_waits(nc)
    return nc


_CACHE = {}


def kernel(**inputs):
    import os

    if "nc" not in _CACHE:
        _CACHE["nc"] = _build()
    nc = _CACHE["nc"]

    f32 = np.float32
    bf16 = ml_dtypes.bfloat16

    def relay(w, ki=128):
        """[N*ki, M] -> [ki, N, M] contiguous bf16."""
        n = w.shape[0] // ki
        return np.ascontiguousarray(
            w.reshape(n, ki, w.shape[1]).transpose(1, 0, 2).astype(bf16)
        )

    src = np.asarray(inputs["src"], f32)
    distances = np.asarray(inputs["distances"], f32)
    scale = np.float32(HD ** -0.5)
    Wq_s = np.asarray(inputs["Wq"], f32) * scale
    bq_s = np.asarray(inputs["bq"], f32) * scale
    Wk_f = np.asarray(inputs["Wk"], f32)
    Wv_f = np.asarray(inputs["Wv"], f32)
    Wo_f = np.asarray(inputs["Wo"], f32)
    g1_f = np.asarray(inputs["g1"], f32)
    beta1_f = np.asarray(inputs["beta1"], f32)
    # LN1 affine folded into the FFN: x = g1*xhat + beta1 =>
    #   relu(x@W1+b1) = relu(xhat@(g1[:,None]*W1) + (b1 + beta1@W1))
    #   and the +x residual becomes +g1*xhat with beta1 folded into b2.
    W1_f = np.asarray(inputs["W1"], f32) * g1_f[:, None]
    b1_f = np.asarray(inputs["b1"], f32) + beta1_f @ np.asarray(inputs["W1"], f32)
    W2_f = np.asarray(inputs["W2"], f32)
    b2_f = np.asarray(inputs["b2"], f32) + beta1_f

    # chunked relayouts: [ki, ko, cols]
    wk_r = np.stack([relay(Wk_f[:, wc * 256 : wc * 256 + 256]) for wc in range(4)])
    wq_r = np.stack([relay(Wq_s[:, wc * 256 : wc * 256 + 256]) for wc in range(4)])
    wv_r = np.stack([relay(Wv_f[:, vc * 512 : vc * 512 + 512]) for vc in range(2)])
    wo_r = relay(Wo_f)
    w1_r = np.stack([relay(W1_f[:, fc * 256 : fc * 256 + 256]) for fc in range(16)])
    w2_r = np.stack([relay(W2_f[:, pq * 256 : pq * 256 + 256]) for pq in range(4)])

    c3 = np.zeros((128, 48), f32)
    c3[:, 0:8] = bq_s.reshape(8, 128).T
    c3[:, 8:16] = np.asarray(inputs["bk"], f32).reshape(8, 128).T
    c3[:, 16:48] = b1_f.reshape(32, 128).T

    rep = lambda v: np.ascontiguousarray(
        np.broadcast_to(np.asarray(v, f32).astype(bf16)[None, :], (128, D))
    )

    shared = {
        "wk": wk_r, "wq": wq_r, "wv": wv_r, "wo": wo_r, "w1": w1_r, "w2": w2_r,
        "c3": c3,
        "bv_r": np.asarray(inputs["bv"], f32).reshape(1, D).copy(),
        "b2_r": b2_f.reshape(1, D).copy(),
        "g1b": rep(inputs["g1"]),
        "g2b": rep(inputs["g2"]),
        "beta2b": rep(inputs["beta2"]),
        "ident": np.eye(128, dtype=f32),
        "ones_row": np.ones((1, 512), f32),
    }

    negabs = -abs(float(np.asarray(inputs["dist_scale"])))
    bo = np.asarray(inputs["bo"], f32)

    in_maps = []
    for c in range(NCORES):
        b, qh = c // 2, c % 2
        q0 = qh * SQ
        if qh == 0:
            perm = np.arange(S)
        else:
            perm = np.r_[np.arange(512, 1024), np.arange(0, 512)]
        m = dict(shared)
        srcTb = src[b][perm].T.astype(bf16)                       # [d, s]
        m["srcT"] = np.ascontiguousarray(
            srcTb.reshape(8, 128, S).transpose(1, 0, 2)
        )
        sq = (src[b, q0 : q0 + SQ] + bo[None, :]).astype(f32)     # [q, d]
        m["src_q"] = np.ascontiguousarray(
            sq.reshape(4, 128, D).transpose(1, 0, 2)
        )
        dT = distances[b, q0 : q0 + SQ][:, perm].T                # [k, q]
        p = np.exp(negabs * np.log(dT + 1e-9)).astype(bf16)
        m["pt"] = np.ascontiguousarray(p.reshape(8, 128, SQ).transpose(1, 0, 2))
        in_maps.append(m)

    trace = bool(int(os.environ.get("BASS_KERNEL_TRACE", "0")))
    res = run_bass_kernel_spmd(
        nc,
        in_maps,
        core_ids=list(range(NCORES)),
        trace=trace,
        stitch_traces=False,
    )
    _CACHE["last_result"] = res

    out = np.empty((B, S, D), f32)
    for c in range(NCORES):
        b, qh = c // 2, c % 2
        out[b, qh * SQ : qh * SQ + SQ] = res.results[c]["out"]
    return out
